# revision 24
# baseline (speedup 1.0000x reference)
"""Trainium2 Bass kernel for nn_CrossAttention_14207751815513.

Single-query cross-attention:
    q = x1 @ Wq.T                 (one query per head)
    k = x2 @ Wk.T ; v = x2 @ Wv.T
    attn_h = softmax(q_h . k_h / sqrt(128))
    out = concat_h(attn_h @ v_h) @ Wo.T + bo

Because there is exactly ONE query, the K and V projections collapse
algebraically (associativity):
    scores_h = x2 @ r_h,  r_h = Wk_h.T q_h / sqrt(128)   -- no k materialization
    out_h    = Wv_h @ (x2.T p_h) / l_h                   -- no v materialization
with p = exp(scores - 2) (shift keeps p <= ~55) and l_h = sum_s p_h[s].

Sharding: the sequence dim (16384) is split across the 8 NeuronCores
(2048 rows each).  Every O(1)-in-sequence quantity (q, R, Wv matvec,
Wo + bias) lives in host-side prep/merge glue; the O(S*C) work runs on
device.

fp8 scheme (vs an all-bf16 version: halves HBM traffic 16->8MB and cuts
PE work ~2x; simulated end-to-end rel_max error 1.09e-2, budget 2e-2):
  - phase S streams x2t as fp8 E3M4 (4 mantissa bits) against a bf16
    stationary R (mixed-dtype matmul); fp32 PSUM accumulation.  x2t is
    prescaled by 2 (fewer subnormals), R carries the /2.
  - x2t is packed in s-major slabs of 512 columns (one PSUM bank each),
    so each bank's accumulation chain stops as soon as its slab is in
    and the exp/P-split/transpose chain for bank m overlaps the matmuls
    of bank m+1.
  - exp via ScalarE activation with bias=-2 (softmax shift-invariant);
    accum_out gives l = sum_s P for free.
  - P splits into Pa + Pb (fp8 E4M3 value + residual, near-bf16 fidelity)
    which are PE-transposed into ONE stacked stationary PT[:, sc, 0:32]
    ([Pa | Pb]), so each DoubleRow phase-T matmul contracts an sc pair
    for BOTH halves at once: out[0:16]=Pa part, out[16:32]=Pb part,
    summed by DVE at the end.  t = (Pa+Pb)^T @ x2n, x2n fp8 E4M3.
  - DoubleRow start=True resets the WHOLE PSUM bank, so only the first
    chain per bank starts; the second half-bank chain rides that reset
    (skip_group_check).

Sync-wait discipline (1 sync wait per engine instruction, no
redundant-wait elision): every input DMA writes a fresh buffer so its
only wait is HW-DGE slot recycling; wait-absorbing PE nops observe the
rsb DMA, the identity, and the last PT copy; ps_t0/1 live in free PSUM
banks and ps_t2/3 reuse the transpose pool's banks so no phase-T matmul
ever combines a WAR wait with a DMA wait; the output DMA goes out on
the gpsimd SWDGE queue with all producers on DVE (single RAW wait).
"""

import sys

for _p in ("/root/.axon_site/_ro/trn_rl_repo", "/opt/trn_rl_repo"):
    if _p not in sys.path:
        sys.path.append(_p)

import numpy as np
import ml_dtypes

import concourse.bass as bass
import concourse.tile as tile
from concourse import mybir
from concourse.bass_utils import run_bass_kernel_spmd
from concourse.tile_rust import add_dep_helper

NCORES = 8
S_FULL = 16384
C = 2048           # input feature dim (both x1 and x2)
H = 16             # heads
J = 128            # head dim (K_DIM == V_DIM == 128)
HJ = H * J         # 2048
ODIM = 512
S_LOC = S_FULL // NCORES   # 2048 sequence rows per core

BF = mybir.dt.bfloat16
F32 = mybir.dt.float32
E3 = mybir.dt.float8e3
E4 = mybir.dt.float8e4
INV_SQRT_K = 1.0 / float(np.sqrt(128.0))
SHIFT = 2.0        # score shift before exp (cancels in t/l)

NB = 512                    # PSUM bank free-dim (f32 columns)
CH = C // 128               # 16 chunks of 128 along any 2048 dim
NPAIR = CH // 2             # 8 sc chunk pairs

_E3_NP = ml_dtypes.float8_e3m4
_E4_NP = ml_dtypes.float8_e4m3
_BF_NP = ml_dtypes.bfloat16

# x2t pieces: (bank, cc_lo, n_cc) -- slab-major, first pieces small so the
# PE starts early.  x2n pieces: (pair_lo, n_pairs).
XT_PIECES = [(0, 0, 4), (0, 4, 4), (0, 8, 8),
             (1, 0, 8), (1, 8, 8),
             (2, 0, 8), (2, 8, 8),
             (3, 0, 8), (3, 8, 8)]
XT_Q = ["a", "s", "g", "a", "s", "g", "a", "s", "g"]
XN_PIECES = [(0, 2), (2, 3), (5, 3)]
XN_Q = ["s", "a", "g"]


def _build_program() -> bass.Bass:
    nc = bass.Bass()
    # x2t host layout [p, bank, cc, col]: slab-major so one bank's full
    # contraction arrives first; x2n [p, pair, r, k, col]: each DoubleRow
    # rhs slice [:, pr, r, :, :] is 512 contiguous bytes per partition.
    t_in = {
        "rsb": nc.dram_tensor("rsb", [J, CH, H], BF, kind="ExternalInput"),
        "x2t": nc.dram_tensor("x2t", [J, 4, CH, NB], E3, kind="ExternalInput"),
        "x2n": nc.dram_tensor("x2n", [J, NPAIR, 8, 2, 256], E4, kind="ExternalInput"),
    }
    # tt rows 0:16 = Pa-part (+l in last col), rows 16:32 = Pb-part; the
    # host merge sums the halves (engines cannot read PSUM at partition
    # base 16, so the add happens off-device)
    t_out = {
        "tt": nc.dram_tensor("tt", [2 * H, C + 1], F32, kind="ExternalOutput"),
    }
    rsb_d = t_in["rsb"][:, :, :]
    x2t_v = t_in["x2t"][:, :, :, :]
    x2n_v = t_in["x2n"][:, :, :, :, :]
    tt_out = t_out["tt"][:, :]

    with tile.TileContext(nc) as tc:
        with (
            tc.tile_pool(name="singles", bufs=1) as singles,
            tc.tile_pool(name="sa", bufs=1) as sa,
            tc.tile_pool(name="sb", bufs=1) as sbp,
            tc.tile_pool(name="psbig", bufs=4, space="PSUM") as psbig,
            tc.tile_pool(name="pstr", bufs=2, space="PSUM") as pstr,
            tc.tile_pool(name="psfree", bufs=1, space="PSUM") as psfree,
        ):
            ep_targets = []  # one representative instruction per proc

            # ---- constants (before DMAs so their engines touch them early)
            bias_t = singles.tile([H, 1], F32)
            nc.vector.memset(bias_t, -SHIFT)
            ident16 = singles.tile([H, H], E4)
            nc.gpsimd.memset(ident16, 0.0)
            i_ident = nc.gpsimd.affine_select(
                out=ident16,
                in_=ident16,
                compare_op=mybir.AluOpType.not_equal,
                fill=1.0,
                base=0,
                pattern=[[-1, H]],
                channel_multiplier=1,
            )

            # ---- issue every stream DMA up front (3 queues) ----------------
            # rsb rides first on sync while the first x2t piece rides first
            # on scalar -- the two legs of the first matmul's critical path
            # transfer in parallel
            q_eng = {"s": nc.sync, "a": nc.scalar, "g": nc.gpsimd}
            Rsb = singles.tile([J, CH, H], BF)
            i_rsb = nc.sync.dma_start(out=Rsb, in_=rsb_d)
            ep_targets.append(i_rsb)

            xt_tiles = {}
            for i, (b, lo, n) in enumerate(XT_PIECES):
                xt = sa.tile([J, n, NB], E3, tag=f"xt{i}", name=f"xt{i}")
                ep_targets.append(
                    q_eng[XT_Q[i]].dma_start(out=xt, in_=x2t_v[:, b, lo : lo + n, :])
                )
                if i == 0:
                    # scalar pre-consumes the DVE bias memset so the
                    # activations inherit it via program order
                    scr = singles.tile([H, 1], F32)
                    nc.scalar.copy(out=scr, in_=bias_t)
                for cc in range(lo, lo + n):
                    xt_tiles[(b, cc)] = (xt, cc - lo)
            xn_tiles = {}
            for i, (lo, n) in enumerate(XN_PIECES):
                xn = sbp.tile([J, n, 8, 2, 256], E4, tag=f"xn{i}", name=f"xn{i}")
                ep_targets.append(
                    q_eng[XN_Q[i]].dma_start(out=xn, in_=x2n_v[:, lo : lo + n, :, :, :])
                )
                for p in range(lo, lo + n):
                    xn_tiles[p] = (xn, p - lo)

            # wait-absorbing PE nops (1 sync wait per instruction max)
            n_rsb = nc.tensor.nop(nofuse=True, hint="dep")
            add_dep_helper(n_rsb.ins, i_rsb.ins, reason="rsb-wait")
            n_id = nc.tensor.nop(nofuse=True, hint="dep")
            add_dep_helper(n_id.ins, i_ident.ins, reason="ident-wait")
            add_dep_helper(n_id.ins, n_rsb.ins, reason="order")

            # ---- phase S + per-bank exp/split/transpose chain --------------
            # scores[h, s] = sum_c R[c, h] x2t[c, s], bank-major (one slab
            # per PSUM bank); the P chain for bank m is emitted after the
            # matmuls of bank m+1 so it overlaps them on the other engines.
            ps_s = [
                psbig.tile([H, NB], F32, tag="big", name=f"ps_s{m}")
                for m in range(4)
            ]
            Pa = singles.tile([H, S_LOC], E4)
            Pb = singles.tile([H, S_LOC], E4)
            Pf = singles.tile([H, S_LOC], F32)
            la = singles.tile([H, 4], F32)
            # stacked stationary: PT[:, sc, 0:16] = Pa^T, [:, sc, 16:32] = Pb^T
            PT = singles.tile([J, CH, 2 * H], E4)

            first_mm = None
            i_act = None
            i_dve = None

            def p_chain(m):
                nonlocal i_act, i_dve
                sl = slice(m * NB, (m + 1) * NB)
                i_act = nc.scalar.activation(
                    out=Pf[:, sl],
                    in_=ps_s[m][:H, :],
                    func=mybir.ActivationFunctionType.Exp,
                    bias=bias_t,
                    accum_out=la[:, m : m + 1],
                )
                nc.vector.tensor_copy(out=Pa[:, sl], in_=Pf[:, sl])
                nc.vector.tensor_tensor(
                    out=Pb[:, sl],
                    in0=Pf[:, sl],
                    in1=Pa[:, sl],
                    op=mybir.AluOpType.subtract,
                )

            def tr_chain(m):
                nonlocal i_dve
                for sb in range(4):
                    sc = 4 * m + sb
                    col = slice(sc * 128, (sc + 1) * 128)
                    # fp8 PE transpose needs output element step of 2
                    psa = pstr.tile([J, H, 2], E4, tag="tr", name=f"psa{sc}")
                    nc.tensor.transpose(psa[:, :, 0], Pa[:, col], ident16)
                    i_dve = nc.vector.tensor_copy(
                        out=PT[:, sc, 0:H], in_=psa[:, :, 0]
                    )
                    psb = pstr.tile([J, H, 2], E4, tag="tr", name=f"psb{sc}")
                    nc.tensor.transpose(psb[:, :, 0], Pb[:, col], ident16)
                    i_dve = nc.vector.tensor_copy(
                        out=PT[:, sc, H : 2 * H], in_=psb[:, :, 0]
                    )

            for m in range(4):
                for cc in range(CH):
                    xt, g = xt_tiles[(m, cc)]
                    i_mm = nc.tensor.matmul(
                        ps_s[m][:H, :],
                        lhsT=Rsb[:, cc, :],
                        rhs=xt[:, g, :],
                        start=(cc == 0),
                        stop=(cc == CH - 1),
                    )
                    if first_mm is None:
                        first_mm = i_mm
                        add_dep_helper(first_mm.ins, n_id.ins, reason="order")
                p_chain(m)
                if m > 0:
                    tr_chain(m - 1)
            tr_chain(3)

            # ---- l = sum_m la[:, m] (matches fp32 P by accum_out) ----------
            l01 = singles.tile([H, 2], F32)
            nc.vector.tensor_tensor(
                out=l01[:, 0:1], in0=la[:, 0:1], in1=la[:, 1:2],
                op=mybir.AluOpType.add,
            )
            nc.vector.tensor_tensor(
                out=l01[:, 1:2], in0=la[:, 2:3], in1=la[:, 3:4],
                op=mybir.AluOpType.add,
            )
            lsum = singles.tile([H, 1], F32)
            nc.vector.tensor_tensor(
                out=lsum, in0=l01[:, 0:1], in1=l01[:, 1:2],
                op=mybir.AluOpType.add,
            )

            # ---- phase T: t[h, c] = sum_s (Pa+Pb)[s, h] x2n[s, c] ----------
            # n_ptc observes the last PT copy so every phase-T matmul's PT
            # read-dep is dominance-elided.
            n_ptc = nc.tensor.nop(nofuse=True, hint="dep")
            add_dep_helper(n_ptc.ins, i_dve.ins, reason="ptcopy-funnel")
            ps_t = [
                psfree.tile([2 * H, NB], F32, tag=f"tf{m}", name=f"ps_t{m}")
                for m in range(2)
            ] + [
                pstr.tile([2 * H, NB], F32, tag="tr", name=f"ps_t{m}")
                for m in range(2, 4)
            ]
            first_t = None
            for p in range(NPAIR):
                xn, pl = xn_tiles[p]
                for r in range(8):
                    m, half = divmod(r, 2)
                    # start=True resets the WHOLE PSUM bank: only half 0
                    # starts, half 1 rides the reset (probe_dr2.py)
                    i_pe = nc.tensor.matmul(
                        ps_t[m][: 2 * H, half * 256 : (half + 1) * 256],
                        lhsT=PT[:, 2 * p : 2 * p + 2, :],
                        rhs=xn[:, pl, r, :, :],
                        start=(p == 0 and half == 0),
                        stop=(p == NPAIR - 1),
                        perf_mode=mybir.MatmulPerfMode.DoubleRow,
                        skip_group_check=(half == 1),
                    )
                    if first_t is None:
                        first_t = i_pe
                        add_dep_helper(i_pe.ins, n_ptc.ins, reason="order")

            # ---- copy out both halves on scalar; the output streams out in
            # per-bank SWDGE pieces as each bank's copy completes (l rides
            # in bank 3's piece, written early)
            tt_sb = singles.tile([2 * H, C + 1], F32)
            i_adds = [nc.scalar.copy(out=tt_sb[0:H, C : C + 1], in_=lsum)]
            i_outs = []
            for m in range(4):
                i_adds.append(
                    nc.scalar.copy(
                        out=tt_sb[:, m * NB : (m + 1) * NB],
                        in_=ps_t[m][: 2 * H, :],
                    )
                )
                if m < 3:
                    i_outs.append(
                        nc.gpsimd.dma_start(
                            out=tt_out[:, m * NB : (m + 1) * NB],
                            in_=tt_sb[:, m * NB : (m + 1) * NB],
                        )
                    )
                else:
                    i_outs.append(
                        nc.gpsimd.dma_start(
                            out=tt_out[:, 3 * NB : C + 1],
                            in_=tt_sb[:, 3 * NB : C + 1],
                        )
                    )
            i_out = i_outs[-1]

            # ---- drain-funnel epilogue ------------------------------------
            ep_targets += [i_ident, i_act, i_dve, i_pe, *i_adds, *i_outs]
            for t in ep_targets:
                n = nc.sync.nop(nofuse=True, hint="dep")
                add_dep_helper(n.ins, t.ins, reason="drain-funnel")

    return nc


_NC_CACHE = None


def _get_nc() -> bass.Bass:
    global _NC_CACHE
    if _NC_CACHE is None:
        _NC_CACHE = _build_program()
    return _NC_CACHE


def _prep_in_maps(x1, x2, Wq, Wk):
    x1 = np.asarray(x1, np.float32)
    x2 = np.asarray(x2, np.float32)
    Wq = np.asarray(Wq, np.float32)
    Wk = np.asarray(Wk, np.float32)

    # R[c, h] = sum_j Wk[h*128+j, c] q[h*128+j] / sqrt(128); /2 absorbs the
    # x2t prescale below
    q = (Wq @ x1) * INV_SQRT_K
    R = np.einsum("hj,hjc->ch", q.reshape(H, J), Wk.reshape(H, J, C)) * 0.5
    rsb = np.ascontiguousarray(
        R.reshape(CH, 128, H).transpose(1, 0, 2)
    ).astype(_BF_NP)                                            # [128, 16, 16]

    in_maps = []
    for c in range(NCORES):
        shard = x2[c * S_LOC : (c + 1) * S_LOC]                 # [2048, 2048]
        # x2t[p, bank, cc, col] = 2*shard.T[cc*128+p, bank*512+col] in E3M4
        # (prescale keeps more values out of the subnormal range; max ~11)
        x2t_f = (shard.T * np.float32(2.0)).reshape(CH, 128, 4, NB)
        x2t_c = np.ascontiguousarray(x2t_f.transpose(1, 2, 0, 3)).astype(_E3_NP)
        # x2n[p, pair, r, k, col] = shard[(2*pair+k)*128+p, r*256+col] in E4M3
        x2n_f = shard.reshape(NPAIR, 2, 128, 8, 256)
        x2n_c = np.ascontiguousarray(x2n_f.transpose(2, 0, 3, 1, 4)).astype(_E4_NP)
        in_maps.append({"rsb": rsb, "x2t": x2t_c, "x2n": x2n_c})
    return in_maps


def _merge(results, Wv, Wo, bo):
    Wv = np.asarray(Wv, np.float32)
    Wo = np.asarray(Wo, np.float32)
    bo = np.asarray(bo, np.float32)
    t_tot = np.zeros((H, C), np.float64)
    l_tot = np.zeros(H, np.float64)
    for r in results:
        tt = r["tt"].astype(np.float64)
        t_tot += tt[:H, :C] + tt[H:, :C]      # Pa part + Pb part
        l_tot += tt[:H, C]
    tn = t_tot / l_tot[:, None]                                 # [16, 2048]
    u = np.einsum("hc,hjc->hj", tn, Wv.astype(np.float64).reshape(H, J, C))
    out = u.reshape(HJ) @ Wo.T.astype(np.float64) + bo.astype(np.float64)
    return out.astype(np.float32).reshape(1, ODIM)


def kernel(x1, x2, Wq, Wk, Wv, Wo, bo):
    nc = _get_nc()
    in_maps = _prep_in_maps(x1, x2, Wq, Wk)
    res = run_bass_kernel_spmd(nc, in_maps, list(range(NCORES)))
    return _merge(res.results, Wv, Wo, bo)


def run_traced(x1, x2, Wq, Wk, Wv, Wo, bo, **trace_kwargs):
    """Like kernel() but returns (output, BassKernelResults) with NTFF trace."""
    nc = _get_nc()
    in_maps = _prep_in_maps(x1, x2, Wq, Wk)
    res = run_bass_kernel_spmd(
        nc, in_maps, list(range(NCORES)), trace=True, **trace_kwargs
    )
    return _merge(res.results, Wv, Wo, bo), res
